# revision 11
# baseline (speedup 1.0000x reference)
"""TLSTM (time-aware LSTM) scan + gather + MLP head for Trainium2, 8-core data parallel.

Model (per reference):
  per step t:  g = 1/log(e+t);  cs = tanh(c@Wd+bd);  c_adj = c + cs*(g-1)
               z = x_t@W + h@U + b;  i,f,cand,o = split(z); sig/sig/tanh/sig
               c = f*c_adj + i*cand;  h = o*tanh(c)
  out = sigmoid(gelu(h[pos]@W1+b1)@W2+b2)

Device mapping (per core, B_loc=16 of B=128), latency-oriented v2:
  State transposed: [units=128 partitions, batch=16 free]. All-tanh trick:
  sigmoid(z) = (tanh(z/2)+1)/2 with 1/2 folded into W/U columns; carried
  state scaled c'=2c, h'=2h (folded into U and W1).

  The per-step serial chain is minimized:
    U-mms -> tanh(4 gates, one ACT op) -> 3 fused DVE ops -> tanh(c) -> h'
  The cs/Wd decay branch (cs=tanh(c@Wd); q=cs*gm1; c_adj=c+q) runs OFF the
  critical path: its Wd matmul + ACT + 2 DVE ops depend only on c'(t-1),
  which is ready ~600ns before h'(t-1), so they overlap the U/gate phase.

  x@W is hoisted out of the loop: one 256-row matmul per (gate, D-half) per
  16-step PSUM chunk [128, 4, 16, 16] f32 (2 banks, 3 chunks in flight),
  issued every other step so it fills the PE's idle window. U-matmuls
  accumulate per-step on top of the preloaded x@W partial sums.

  Elementwise intermediates are f32 (bf16 DVE/ACT ops add internal rounding
  noise ~50x beyond output rounding); bf16 only for matmul movings (cbf cast
  off-chain, h at its single write). Gather-at-position done arithmetically:
  sel = reduce_t(hist * onehot).
"""

import sys
import numpy as np

if "/opt/trn_rl_repo" not in sys.path:
    sys.path.insert(0, "/opt/trn_rl_repo")

import ml_dtypes

BF16 = ml_dtypes.bfloat16

B, T, D = 128, 1024, 256
UNITS, HID, OUT = 128, 64, 8
NCORES = 8
BL = B // NCORES  # 16 per-core batch
TS = 16           # timesteps per PSUM chunk


def build_module(Tn=T, slow_bias=False):
    from contextlib import ExitStack

    import concourse.bass as bass
    import concourse.tile as tile
    from concourse import mybir
    from concourse.bacc import Bacc

    assert Tn % TS == 0
    NCHUNK = Tn // TS

    f32 = mybir.dt.float32
    bf16 = mybir.dt.bfloat16
    AF = mybir.ActivationFunctionType
    OPA = mybir.AluOpType

    nc = Bacc("TRN2", target_bir_lowering=False, debug=False, num_devices=NCORES)

    xT_d = nc.dram_tensor("xT", [D, Tn * BL], bf16, kind="ExternalInput")
    gm1_d = nc.dram_tensor("gm1", [128, Tn, BL], bf16, kind="ExternalInput")
    oh_d = nc.dram_tensor("oh", [128, Tn, BL], bf16, kind="ExternalInput")
    Wp_d = nc.dram_tensor("Wp", [D, 4 * UNITS], bf16, kind="ExternalInput")
    Up_d = nc.dram_tensor("Up", [UNITS, 4 * UNITS], bf16, kind="ExternalInput")
    Wd_d = nc.dram_tensor("Wdp", [UNITS, UNITS], bf16, kind="ExternalInput")
    W1_d = nc.dram_tensor("W1p", [UNITS, HID], bf16, kind="ExternalInput")
    W2_d = nc.dram_tensor("W2p", [HID, OUT], bf16, kind="ExternalInput")
    b1_d = nc.dram_tensor("b1v", [HID, 1], f32, kind="ExternalInput")
    b2_d = nc.dram_tensor("b2v", [OUT, 1], f32, kind="ExternalInput")
    if slow_bias:
        # per-gate bias rows added into each PSUM chunk via a K=1 matmul,
        # bd handled as per-partition ACT bias on the cs tanh.
        bias4_d = nc.dram_tensor("bias4", [4, UNITS], bf16, kind="ExternalInput")
        bdv_d = nc.dram_tensor("bdv", [UNITS, 1], f32, kind="ExternalInput")
    out_d = nc.dram_tensor("outT", [OUT, BL], f32, kind="ExternalOutput")

    with tile.TileContext(nc) as tc, ExitStack() as ctx:
        singles = ctx.enter_context(tc.tile_pool(name="singles", bufs=1))
        tmp = ctx.enter_context(tc.tile_pool(name="tmp", bufs=3))
        cpool = ctx.enter_context(tc.tile_pool(name="cpool", bufs=3))
        zpsum = ctx.enter_context(tc.tile_pool(name="zps", bufs=3, space="PSUM"))
        cspsum = ctx.enter_context(tc.tile_pool(name="csps", bufs=2, space="PSUM"))
        hpsum = ctx.enter_context(tc.tile_pool(name="hps", bufs=1, space="PSUM"))

        # ---- resident SBUF tensors --------------------------------------
        xt_s = [singles.tile([128, Tn, BL], bf16, tag=f"xt{h}", name=f"xt{h}") for h in range(2)]
        gm1_s = singles.tile([128, Tn, BL], bf16)
        oh_s = singles.tile([128, Tn, BL], bf16)
        hist = singles.tile([128, Tn, BL], bf16, tag="hist", name="hist")
        w_s = [singles.tile([128, 4 * UNITS], bf16, tag=f"w{h}", name=f"w{h}") for h in range(2)]
        u_s = singles.tile([UNITS, 4 * UNITS], bf16)
        wd_s = singles.tile([UNITS, UNITS], bf16)
        w1_s = singles.tile([UNITS, HID], bf16)
        w2_s = singles.tile([HID, OUT], bf16)
        b1_s = singles.tile([HID, 1], f32)
        b2_s = singles.tile([OUT, 1], f32)
        zero_h = singles.tile([128, BL], bf16)
        zero_cb = singles.tile([128, BL], bf16)
        if slow_bias:
            bias4_s = singles.tile([4, UNITS], bf16)
            ones_s = singles.tile([4, BL], bf16)
            bdv_s = singles.tile([UNITS, 1], f32)

        # ---- input DMAs --------------------------------------------------
        x3 = xT_d.ap().rearrange("d (t b) -> d t b", b=BL)
        for h in range(2):
            nc.sync.dma_start(out=xt_s[h], in_=x3[128 * h : 128 * (h + 1), :, :])
        nc.sync.dma_start(out=gm1_s, in_=gm1_d.ap())
        nc.sync.dma_start(out=oh_s, in_=oh_d.ap())
        for h in range(2):
            nc.sync.dma_start(out=w_s[h], in_=Wp_d.ap()[128 * h : 128 * (h + 1), :])
        nc.sync.dma_start(out=u_s, in_=Up_d.ap())
        nc.sync.dma_start(out=wd_s, in_=Wd_d.ap())
        nc.sync.dma_start(out=w1_s, in_=W1_d.ap())
        nc.sync.dma_start(out=w2_s, in_=W2_d.ap())
        nc.sync.dma_start(out=b1_s, in_=b1_d.ap())
        nc.sync.dma_start(out=b2_s, in_=b2_d.ap())
        if slow_bias:
            nc.sync.dma_start(out=bias4_s, in_=bias4_d.ap())
            nc.sync.dma_start(out=bdv_s, in_=bdv_d.ap())
            nc.vector.memset(ones_s, 1.0)

        nc.vector.memset(zero_h, 0.0)
        nc.vector.memset(zero_cb, 0.0)

        # ---- per-step gate-preact banks ---------------------------------
        # zb(t): [128, 4(gate), BL] f32, one PSUM bank per step (3 rotating).
        # PSUM start_tensor_calc marks the whole 2KB bank pending-zero, so
        # only the FIRST x@W matmul of a step carries start=True (it lazily
        # zeroes the bank; the other gates' first writes land on pending-
        # zero bytes and overwrite). The last U matmul stops the group so
        # the gate tanh may read the bank.
        zbanks = {}

        def zx_mms(t):
            """Issue the 8 x@W matmuls (+bias) for step t into a fresh bank."""
            zb = zpsum.tile([128, 4, BL], f32, tag="zb", name=f"zb{t % 3}")
            zbanks[t] = zb
            for h in range(2):
                for g in range(4):
                    nc.tensor.matmul(
                        zb[:, g],
                        w_s[h][:, 128 * g : 128 * (g + 1)],
                        xt_s[h][:, t, :],
                        start=(h == 0 and g == 0),
                        stop=False,
                    )
            if slow_bias:
                for g in range(4):
                    nc.tensor.matmul(
                        zb[:, g],
                        bias4_s[g : g + 1, :],
                        ones_s[g : g + 1, :],
                        start=False,
                        stop=False,
                    )

        # prologue: fill step 0
        zx_mms(0)

        # ---- scan --------------------------------------------------------
        h_prev = zero_h
        cb_prev = zero_cb
        for t in range(Tn):
            zb = zbanks[t]

            # --- PE: cs-branch matmul first (operand ready earliest) ---
            csb = cspsum.tile([128, BL], f32, tag="csb", name="csb")
            nc.tensor.matmul(csb, wd_s, cb_prev[:], start=True, stop=True)
            # --- PE: U accumulation on top of preloaded x@W sums ---
            # Explicit ldweights for the first U matmul so the weight load
            # runs inside the h'-wait window instead of serializing after it.
            nc.tensor.ldweights(u_s[:, 0:128])
            for g in range(4):
                nc.tensor.matmul(
                    zb[:, g],
                    u_s[:, 128 * g : 128 * (g + 1)],
                    h_prev[:],
                    start=False,
                    stop=(g == 3),
                )
            # --- PE: next step's x@W in the idle window ---
            if t + 1 < Tn:
                zx_mms(t + 1)

            # --- cs branch (off critical path) ---
            cs = tmp.tile([128, BL], f32, tag="cs", name="cs")
            if slow_bias:
                nc.scalar.activation(cs, csb[:], AF.Tanh, scale=0.5, bias=bdv_s[:, 0:1])
            else:
                nc.scalar.activation(cs, csb[:], AF.Tanh, scale=0.5)
            q = tmp.tile([128, BL], f32, tag="q", name="q")
            nc.vector.tensor_mul(q, cs[:], gm1_s[:, t, :])
            c_adj = tmp.tile([128, BL], f32, tag="ca", name="ca")
            nc.vector.tensor_add(c_adj, cb_prev[:], q[:])

            # --- gates: single tanh over the 4 gate groups ---
            S = tmp.tile([128, 4, BL], f32, tag="S", name="S")
            nc.scalar.activation(S, zb[:], AF.Tanh)

            # --- combine: c' = 0.5*(Sf+1)*c_adj + (Si+1)*CD ---
            a2 = tmp.tile([128, BL], f32, tag="a2", name="a2")
            nc.vector.scalar_tensor_tensor(
                a2, S[:, 0, :], 1.0, S[:, 3, :], OPA.add, OPA.mult
            )
            a1 = tmp.tile([128, BL], f32, tag="a1", name="a1")
            nc.vector.scalar_tensor_tensor(
                a1, S[:, 1, :], 1.0, c_adj[:], OPA.add, OPA.mult
            )
            c_new = cpool.tile([128, BL], f32, tag="cn", name="cn")
            nc.vector.scalar_tensor_tensor(
                c_new, a1[:], 0.5, a2[:], OPA.mult, OPA.add
            )

            # --- h' = (So+1) * tanh(c'/2) ---
            tau = tmp.tile([128, BL], f32, tag="tau", name="tau")
            nc.scalar.activation(tau, c_new[:], AF.Tanh, scale=0.5)
            cbf = cpool.tile([128, BL], bf16, tag="cb", name="cb")
            nc.vector.tensor_copy(cbf, c_new[:])
            nc.vector.scalar_tensor_tensor(
                hist[:, t, :], S[:, 2, :], 1.0, tau[:], OPA.add, OPA.mult
            )

            h_prev = hist[:, t, :]
            cb_prev = cbf

        # ---- gather at position + head ----------------------------------
        sel = singles.tile([128, BL], f32)
        m = singles.tile([128, Tn, BL], bf16, tag="m", name="m")
        nc.vector.tensor_mul(m, hist[:], oh_s[:])
        nc.vector.tensor_reduce(
            sel,
            m[:].rearrange("p t b -> p b t"),
            mybir.AxisListType.X,
            OPA.add,
        )
        selb = singles.tile([128, BL], bf16)
        nc.vector.tensor_copy(selb, sel[:])

        ph1 = hpsum.tile([HID, BL], f32, tag="ph1")
        nc.tensor.matmul(ph1, w1_s[:], selb[:], start=True, stop=True)
        y1 = singles.tile([HID, BL], bf16)
        nc.scalar.activation(y1, ph1[:], AF.Gelu, bias=b1_s[:, 0:1])
        ph2 = hpsum.tile([OUT, BL], f32, tag="ph2")
        nc.tensor.matmul(ph2, w2_s[:], y1[:], start=True, stop=True)
        yout = singles.tile([OUT, BL], f32)
        nc.scalar.activation(yout, ph2[:], AF.Sigmoid, bias=b2_s[:, 0:1])
        nc.sync.dma_start(out=out_d.ap(), in_=yout[:])

    nc.finalize()
    return nc


def prep_inputs(x, time, position, W, U, b, Wd, bd, W1, b1, W2, b2, Tn=T):
    """Host-side prep. Returns (in_maps, slow_bias)."""
    x = np.asarray(x, np.float32)[:, :Tn]
    time = np.asarray(time, np.float32)[:, :Tn]
    position = np.asarray(position).astype(np.int64)
    W = np.asarray(W, np.float32)
    U = np.asarray(U, np.float32)
    b = np.asarray(b, np.float32)
    Wd = np.asarray(Wd, np.float32)
    bd = np.asarray(bd, np.float32)
    W1 = np.asarray(W1, np.float32)
    b1 = np.asarray(b1, np.float32)
    W2 = np.asarray(W2, np.float32)
    b2 = np.asarray(b2, np.float32)

    slow_bias = bool(np.any(b != 0) or np.any(bd != 0))

    # reorder gate columns [i f c o] -> [i f o c], apply all-tanh/state scalings
    def perm(M):
        return np.concatenate([M[:, :256], M[:, 384:], M[:, 256:384]], axis=1)

    Wp = perm(W).copy()
    Wp[:, :384] *= 0.5
    Up = perm(U).copy()
    Up[:, :384] *= 0.25
    Up[:, 384:] *= 0.5
    W1p = W1 * 0.5

    bp = np.concatenate([b[:256], b[384:], b[256:384]])
    bias4 = np.stack(
        [bp[0:128] * 0.5, bp[128:256] * 0.5, bp[256:384] * 0.5, bp[384:512]]
    ).astype(np.float32)

    gm1_full = (2.0 * (1.0 / np.log(np.e + time) - 1.0)).astype(np.float32)  # [B,Tn]

    common = {
        "Wp": Wp.astype(BF16),
        "Up": Up.astype(BF16),
        "Wdp": Wd.astype(BF16),
        "W1p": W1p.astype(BF16),
        "W2p": W2.astype(BF16),
        "b1v": b1.reshape(HID, 1).astype(np.float32),
        "b2v": b2.reshape(OUT, 1).astype(np.float32),
    }
    if slow_bias:
        common["bias4"] = bias4.astype(BF16)
        common["bdv"] = (2.0 * bd).reshape(UNITS, 1).astype(np.float32)

    in_maps = []
    for k in range(NCORES):
        sl = slice(BL * k, BL * (k + 1))
        # [D, Tn, BL]
        xT = (
            np.ascontiguousarray(x[sl].transpose(2, 1, 0))
            .reshape(D, Tn * BL)
            .astype(BF16)
        )
        gm1 = np.broadcast_to(
            np.ascontiguousarray(gm1_full[sl].T).astype(BF16)[None, :, :],
            (128, Tn, BL),
        ).copy()
        oh = np.zeros((Tn, BL), np.float32)
        for bb in range(BL):
            p = min(int(position[BL * k + bb]), Tn - 1)
            oh[p, bb] = 1.0
        im = dict(common)
        im["xT"] = xT
        im["gm1"] = gm1
        im["oh"] = np.broadcast_to(oh.astype(BF16)[None, :, :], (128, Tn, BL)).copy()
        in_maps.append(im)
    return in_maps, slow_bias


_CACHE = {}


def run(inputs, Tn=T, trace=False):
    from concourse.bass_utils import run_bass_kernel_spmd

    in_maps, slow_bias = prep_inputs(**inputs, Tn=Tn)
    key = (Tn, slow_bias)
    if key not in _CACHE:
        _CACHE[key] = build_module(Tn, slow_bias)
    nc = _CACHE[key]
    res = run_bass_kernel_spmd(
        nc, in_maps, core_ids=list(range(NCORES)), trace=trace
    )
    out = np.zeros((B, OUT), np.float32)
    for k in range(NCORES):
        out[BL * k : BL * (k + 1)] = np.asarray(
            res.results[k]["outT"], np.float32
        ).T
    return out, res


def kernel(**inputs) -> np.ndarray:
    out, _ = run(inputs, Tn=T, trace=False)
    return out
